# revision 5
# baseline (speedup 1.0000x reference)
"""Trainium2 Bass kernel for nn_CIFARDiffusionLayer (5394478923805).

The reference module is LINEAR in u:
  - every tridiagonal ADI solve has batch-independent coefficients
    (built from the tiny [C,32,32] parameter maps), and
  - einsum('cc,bchw->bchw', coupling, u) with the repeated index is a
    per-channel diagonal scale.
So the whole 4-step loop collapses, per channel, to one dense [1024,1024]
matrix L_c acting on flattened 32x32 images:  out[b,c] = L_c @ vec(u[b,c]).
L_c is built on host in float64 by pushing the 1024 basis vectors through the
exact reference recurrences (including the EPS fudge).  The operator's true
spatial support is |Δrow| <= 4 image rows, which fits exactly in a block-
TRIdiagonal structure with 128-pixel (4-row) chunks (BAND=1; rel err ~1.3e-3
of absmax vs the 2e-2 gate), so the device kernel runs a banded block
matmul — a single data-parallel pass over u (one HBM read + one write = the
memory roofline):

per 128-batch tile (per core, batch-sharded 8 ways):
  ONE contiguous DMA of the tile's pixel-major fp16 block (the host performs
  the batch<->pixel transpose while sharding - an exact relayout that removes
  all on-device transposes)
  -> fp16 banded matmuls (fp32 PSUM accumulate), data stationary / operator
     moving; per channel both 512-col halves accumulate into one 2-bank
     [128,1024] PSUM tile, drained by a single wide ACT or DVE copy
  -> one full-row [128,3072] fp16 store per tile.

Queue split (the previous all-on-sync layout serialized loads behind store
semaphore waits in the single FIFO ring and left the 16 SDMA engines ~31%
idle): u/W loads issue on the sync (SP) HWDGE ring and prefetch 8 tiles
ahead; output stores issue on the scalar (ACT) HWDGE ring right after the
PSUM drains they depend on.

Everything on-device is fp16.  The operator entries are ~1e-4 scale —
fp16-subnormal territory — so the host scales W by 4096 (exact power of two)
and divides the gathered output back.  End-to-end error vs the reference:
~1.3e-3 of output absmax (BAND=1 truncation dominates; fp16 rounding alone
is ~6e-4).
"""
import os
from contextlib import ExitStack

import numpy as np

DT = 0.15
DX = 1.0
NUM_STEPS = 4
EPS = 1e-6
S = 32
C = 3
PIX = S * S          # 1024
KC = PIX // 128      # 8 k-chunks per channel
ROW = C * PIX        # 3072 floats per batch
B_TOTAL = 16384
N_CORES = 8
B_CORE = B_TOTAL // N_CORES
BAND = int(os.environ.get("KERNEL_BAND", "1"))  # block band half-width


def _klist(h):
    """In-band k-chunks for output half h (m-chunks 4h..4h+3)."""
    return list(range(max(0, 4 * h - BAND), min(KC, 4 * h + 3 + BAND + 1)))


def _slices(h):
    """Tight column ranges per in-band k for half h: [(k, col_start, col_end)].

    Only m-chunks within BAND of k are nonzero; fp16 matmuls run 1 cyc/row at
    any width, so ranges are exactly the in-band columns.  Accumulation with
    per-k partial column ranges is safe: the start=True matmul clears the whole
    PSUM bank's has_written bits, so each element's first writer overwrites.
    """
    res = []
    for k in _klist(h):
        mlo = max(4 * h, k - BAND)
        mhi = min(4 * h + 4, k + 1 + BAND)
        res.append((k, (mlo - 4 * h) * 128, (mhi - 4 * h) * 128))
    return res


def _wtot(h):
    return sum(ce - cs for _, cs, ce in _slices(h))


_CACHE = {}
LAST_RESULTS = None  # BassKernelResults of the most recent run (for test.py)


# ----------------------------- host-side operator ---------------------------

def _smooth3(m, axis):
    p = np.concatenate([m.take([0], axis=axis), m, m.take([-1], axis=axis)],
                       axis=axis)
    n = m.shape[axis]
    sl = lambda i: p.take(range(i, i + n), axis=axis)
    return (sl(0) + sl(1) + sl(2)) / 3.0


def _thomas_matrix(a, b, c):
    """Exact linear map of the reference thomas() for one N-system, as [N,N]."""
    N = a.shape[0]
    d = np.eye(N, dtype=np.float64)
    cp = 0.0
    dp = np.zeros(N, dtype=np.float64)
    cs = np.zeros(N, dtype=np.float64)
    ds = np.zeros((N, N), dtype=np.float64)
    for i in range(N):
        denom = b[i] - a[i] * cp + EPS
        cn = c[i] / denom
        dn = (d[i] - a[i] * dp) / denom
        cs[i] = cn
        ds[i] = dn
        cp, dp = cn, dn
    cs[N - 1] = 0.0
    x = np.zeros((N, N), dtype=np.float64)
    xn = np.zeros(N, dtype=np.float64)
    for i in range(N - 1, -1, -1):
        x[i] = ds[i] - cs[i] * xn
        xn = x[i]
    return x


def _solve_matrices(coeff_smooth, dt):
    coeff = coeff_smooth * dt / (DX ** 2)
    a = -coeff
    c = -coeff
    b = 1.0 + 2.0 * coeff
    b = b.copy()
    b[..., 0] = 1.0 + coeff[..., 0]
    b[..., -1] = 1.0 + coeff[..., -1]
    Cn, K, N = a.shape
    out = np.zeros((Cn, K, N, N), dtype=np.float64)
    for ci in range(Cn):
        for k in range(K):
            out[ci, k] = _thomas_matrix(a[ci, k], b[ci, k], c[ci, k])
    return out


def _build_operator(alpha_base, beta_base, alpha_time_coeff, beta_time_coeff,
                    channel_coupling):
    """[C, 1024, 1024] float64: out_vec = L[c] @ u_vec (h*32+w order)."""
    ab = alpha_base.astype(np.float64)
    bb = beta_base.astype(np.float64)
    at = alpha_time_coeff.astype(np.float64)
    bt = beta_time_coeff.astype(np.float64)
    diag = np.diag(channel_coupling.astype(np.float64))

    M = np.broadcast_to(np.eye(PIX, dtype=np.float64).reshape(S, S, PIX),
                        (C, S, S, PIX)).copy()
    t = 0.0
    for _ in range(NUM_STEPS):
        alpha = np.maximum(ab + at * t, EPS)
        beta = np.maximum(bb + bt * t, EPS)
        Sx = _solve_matrices(_smooth3(alpha, axis=2), DT / 2)        # [C,H,w',w]
        bsm = _smooth3(beta, axis=1)
        Sy = _solve_matrices(np.transpose(bsm, (0, 2, 1)), DT)       # [C,W,h',h]
        M = np.einsum('chvw,chwK->chvK', Sx, M)
        M = np.einsum('cwuh,chwK->cuwK', Sy, M)
        M = np.einsum('chvw,chwK->chvK', Sx, M)
        M = M * diag[:, None, None, None]
        t += DT
    return M.reshape(C, PIX, PIX)


# ----------------------------- device program -------------------------------

def _build_program(nc, u_ap, w_ap, id_ap, out_ap, b_per_core):
    import concourse.tile as tile
    from concourse import mybir
    F32 = mybir.dt.float32
    F16 = mybir.dt.float16
    ntiles = b_per_core // 128
    wtot = _wtot(0)

    with tile.TileContext(nc) as tc, ExitStack() as ctx:
        const_pool = ctx.enter_context(tc.tile_pool(name="const", bufs=1))
        w_pool = ctx.enter_context(tc.tile_pool(name="w", bufs=1))
        ut_pool = ctx.enter_context(tc.tile_pool(name="ut", bufs=8))
        out_pool = ctx.enter_context(tc.tile_pool(name="out", bufs=4))
        # all 8 PSUM banks go to the matmul pipeline: 4 bufs x 2 banks.  The
        # warm-up transposes below borrow the same rotating slots.
        psm_pool = ctx.enter_context(tc.tile_pool(name="psm", bufs=4,
                                                  space="PSUM"))

        ident = const_pool.tile([128, 128], F16)
        nc.sync.dma_start(out=ident[:], in_=id_ap[:])
        # u arrives pre-transposed from the host: u_ap[tile, kk, blk*128+b]
        # (pixel-major per 128-batch tile), so each tile is ONE contiguous DMA
        # straight into the matmul operand layout - no PE transposes needed.

        # HAM warm-up: throwaway transposes of the identity bridge the PE
        # from the ident-load landing to the first real matmuls (~1.5us) so
        # the 3.4us activity window that flips the clock-gate to 8/8 starts
        # ticking as early as possible.
        for wi in range(12):
            wp = psm_pool.tile([128, 128], F16, tag="psm", name="warm")
            nc.tensor.transpose(wp[:], ident[:], ident[:])

        # All loads stream in order on the sync (SP) HWDGE ring: u0 first so
        # tile 0 can start, then the per-channel operator slices, then the
        # remaining u tiles (prefetch depth = ut_pool bufs).  Stores go on
        # the scalar (ACT) ring so a store's semaphore wait never blocks a
        # load issue (single-queue FIFO was the old bottleneck).
        u_tiles = [None] * ntiles
        u_tiles[0] = ut_pool.tile([128, ROW], F16, tag="utall", name="utall")
        nc.sync.dma_start(out=u_tiles[0][:], in_=u_ap[0])
        wt = [None] * C
        for c in range(C):
            t = w_pool.tile([128, 2 * wtot], F16, tag=f"w{c}")
            nc.sync.dma_start(out=t[:], in_=w_ap[:, c * 2 * wtot:
                                                 (c + 1) * 2 * wtot])
            wt[c] = t

        def ut_views(utall):
            return [[utall[:, (2 * c + k // 4) * 512 + (k % 4) * 128:
                           (2 * c + k // 4) * 512 + (k % 4 + 1) * 128]
                     for k in range(KC)] for c in range(C)]

        def emit_tile(it, utall):
            ut = ut_views(utall)
            last = it == ntiles - 1
            out_nat = out_pool.tile([128, ROW], F16, name="out_nat")
            for c in range(C):
                # one 2-bank PSUM tile per channel; each 512-col half is its
                # own accumulation group confined to one bank
                ps = psm_pool.tile([128, 1024], F32, tag="psm", name="ps")
                for h in range(2):
                    sl = _slices(h)
                    off = 0
                    for i, (k, cs, ce) in enumerate(sl):
                        nc.tensor.matmul(
                            ps[:, 512 * h + cs:512 * h + ce], lhsT=ut[c][k],
                            rhs=wt[c][:, h * wtot + off:
                                      h * wtot + off + (ce - cs)],
                            start=(i == 0), stop=(i == len(sl) - 1))
                        off += ce - cs
                # psum drain: one wide copy per channel, split ACT/DVE so the
                # two PSUM read ports run in parallel (ACT also issues the
                # store, so DVE takes two of the three channels)
                if c == 0:
                    nc.scalar.copy(out_nat[:, 0:PIX], ps[:])
                else:
                    nc.vector.tensor_copy(
                        out_nat[:, c * PIX:(c + 1) * PIX], ps[:])
                if last:
                    # per-channel stores shorten the epilogue: each channel's
                    # bytes hit the wire as soon as its drain lands
                    nc.scalar.dma_start(
                        out=out_ap[it * 128:(it + 1) * 128,
                                   c * PIX:(c + 1) * PIX],
                        in_=out_nat[:, c * PIX:(c + 1) * PIX])
            if not last:
                nc.scalar.dma_start(
                    out=out_ap[it * 128:(it + 1) * 128, :], in_=out_nat[:])

        for it in range(ntiles):
            if it + 1 < ntiles:
                u_tiles[it + 1] = ut_pool.tile([128, ROW], F16, tag="utall",
                                               name="utall")
                nc.sync.dma_start(out=u_tiles[it + 1][:], in_=u_ap[it + 1])
            emit_tile(it, u_tiles[it])


def _get_nc():
    if "nc" in _CACHE:
        return _CACHE["nc"]
    from concourse import bacc, mybir
    # num_devices=1: the 8 cores are pure SPMD replicas with no collectives,
    # so skip the cross-core EVSEM butterfly in the kernel pre/postamble.
    nd = int(os.environ.get("KERNEL_ND", "1"))
    nc = bacc.Bacc("TRN2", target_bir_lowering=False, debug=False,
                   num_devices=nd)
    F16 = mybir.dt.float16
    wtot = _wtot(0)
    u_ap = nc.dram_tensor("u", [B_CORE // 128, 128, ROW], F16,
                          kind="ExternalInput").ap()
    w_ap = nc.dram_tensor("w", [128, C * 2 * wtot], F16,
                          kind="ExternalInput").ap()
    id_ap = nc.dram_tensor("ident", [128, 128], F16,
                           kind="ExternalInput").ap()
    out_ap = nc.dram_tensor("out", [B_CORE, ROW], F16,
                            kind="ExternalOutput").ap()
    _build_program(nc, u_ap, w_ap, id_ap, out_ap, B_CORE)
    nc.compile()
    _CACHE["nc"] = nc
    return nc


def _inject_ntff_hook():
    import sys, types
    try:
        import antenv.axon_hooks  # noqa: F401
        return
    except ImportError:
        pass
    from trn_agent_boot.trn_boot import _ntff_profile_via_ctypes
    hook = _ntff_profile_via_ctypes('/opt/axon/libaxon_pjrt.so')
    mod = types.ModuleType('antenv.axon_hooks')
    _state = {'hook': hook}
    mod.get_axon_ntff_profile_hook = lambda: _state['hook']
    mod.set_axon_ntff_profile_hook = lambda h: _state.update(hook=h)
    sys.modules['antenv.axon_hooks'] = mod
    import antenv
    antenv.axon_hooks = mod


# ----------------------------- entry point ----------------------------------

def kernel(u, alpha_base, beta_base, alpha_time_coeff, beta_time_coeff,
           channel_coupling):
    global LAST_RESULTS
    u = np.asarray(u, dtype=np.float32)
    assert u.shape == (B_TOTAL, C, S, S), u.shape

    L = _build_operator(np.asarray(alpha_base), np.asarray(beta_base),
                        np.asarray(alpha_time_coeff),
                        np.asarray(beta_time_coeff),
                        np.asarray(channel_coupling))
    # tight-packed banded moving-operand slices, concatenated along free dim
    wtot = _wtot(0)
    w = np.zeros((C, 2, 128, wtot), dtype=np.float32)
    LT = L.transpose(0, 2, 1).astype(np.float32)  # [c, kpix, npix]
    for h in range(2):
        off = 0
        for k, cs, ce in _slices(h):
            w[:, h, :, off:off + (ce - cs)] = \
                LT[:, k * 128:(k + 1) * 128, 512 * h + cs:512 * h + ce]
            off += ce - cs
    # x4096 (exact power of 2) lifts the ~1e-4-scale operator entries out of
    # fp16's subnormal zone; the host divides the output back
    w = (w * 4096.0).astype(np.float16)
    # partition-major relayout: w2[p, (c,h,off)] so each channel's slices are
    # one contiguous [128, 2*wtot] DMA
    w2 = np.ascontiguousarray(w.transpose(2, 0, 1, 3)).reshape(
        128, C * 2 * wtot)
    ident = np.eye(128, dtype=np.float16)

    nc = _get_nc()
    from concourse import bass_utils

    # pixel-major per 128-batch tile: u_t[tile, kk, blk*128 + b]
    u16 = u.reshape(B_TOTAL // 128, 128, ROW // 128, 128).astype(np.float16)
    u2 = np.ascontiguousarray(u16.transpose(0, 3, 2, 1)).reshape(
        B_TOTAL // 128, 128, ROW)
    tpc = B_CORE // 128
    in_maps = [{"u": u2[i * tpc:(i + 1) * tpc], "w": w2, "ident": ident}
               for i in range(N_CORES)]

    trace = os.environ.get("KERNEL_TRACE", "") == "1"
    kw = {}
    if trace:
        _inject_ntff_hook()
        bass_utils.upload_artifacts = lambda tmpdir: tmpdir
        kw = dict(trace=True, tmpdir=os.environ.get("KERNEL_TRACE_DIR"))

    # Expected result for one batch row per core, for output verification
    # (the devices occasionally fail transiently — exceptions AND, rarely,
    # silently corrupted buffers — so verify and retry).
    uf0 = u.reshape(B_TOTAL, C, PIX)
    checks = []
    for i in range(N_CORES):
        b = i * B_CORE
        checks.append(np.concatenate(
            [L[c] @ uf0[b, c].astype(np.float64) for c in range(C)]))

    import time
    last_exc = None
    for attempt in range(3):
        try:
            if trace:
                # stale ntffs from a prior run/attempt make the profiler
                # abort (parallel-instances assert) — start from a clean dir
                tdir = os.environ.get("KERNEL_TRACE_DIR")
                if tdir and os.path.isdir(tdir):
                    for f in os.listdir(tdir):
                        try:
                            os.remove(os.path.join(tdir, f))
                        except OSError:
                            pass
            res = bass_utils.run_bass_kernel_spmd(
                nc, in_maps, core_ids=list(range(N_CORES)), **kw)
        except Exception as e:
            last_exc = e
            time.sleep(5)
            continue
        ok = True
        for i in range(N_CORES):
            got = res.results[i]["out"][0].astype(np.float64) / 4096.0
            ref = checks[i]
            tol = 0.05 * max(np.abs(ref).max(), 1e-30)
            if not np.all(np.isfinite(got)) or np.abs(got - ref).max() > tol:
                ok = False
                break
        if ok:
            break
        time.sleep(5)
    else:
        if last_exc is not None:
            raise last_exc
    LAST_RESULTS = res

    out = np.concatenate([r["out"] for r in res.results], axis=0)
    out = out.astype(np.float32) * (1.0 / 4096.0)
    return out.reshape(B_TOTAL, C, S, S)


# revision 7
# speedup vs baseline: 1.0024x; 1.0024x over previous
"""Trainium2 Bass kernel for nn_CIFARDiffusionLayer (5394478923805).

The reference module is LINEAR in u:
  - every tridiagonal ADI solve has batch-independent coefficients
    (built from the tiny [C,32,32] parameter maps), and
  - einsum('cc,bchw->bchw', coupling, u) with the repeated index is a
    per-channel diagonal scale.
So the whole 4-step loop collapses, per channel, to one dense [1024,1024]
matrix L_c acting on flattened 32x32 images:  out[b,c] = L_c @ vec(u[b,c]).
L_c is built on host in float64 by pushing the 1024 basis vectors through the
exact reference recurrences (including the EPS fudge).  The operator's true
spatial support is |Δrow| <= 4 image rows, which fits exactly in a block-
TRIdiagonal structure with 128-pixel (4-row) chunks (BAND=1; rel err ~1.3e-3
of absmax vs the 2e-2 gate), so the device kernel runs a banded block
matmul — a single data-parallel pass over u (one HBM read + one write = the
memory roofline):

per 128-batch tile (per core, batch-sharded 8 ways):
  ONE contiguous DMA of the tile's pixel-major fp16 block (the host performs
  the batch<->pixel transpose while sharding - an exact relayout that removes
  all on-device transposes)
  -> fp16 banded matmuls (fp32 PSUM accumulate), data stationary / operator
     moving; per channel both 512-col halves accumulate into one 2-bank
     [128,1024] PSUM tile, drained by a single wide ACT or DVE copy
  -> one full-row [128,3072] fp16 store per tile.

Queue split (the previous all-on-sync layout serialized loads behind store
semaphore waits in the single FIFO ring and left the 16 SDMA engines ~31%
idle): u/W loads issue on the sync (SP) HWDGE ring and prefetch 8 tiles
ahead; output stores issue on the scalar (ACT) HWDGE ring right after the
PSUM drains they depend on.

Everything on-device is fp16.  The operator entries are ~1e-4 scale —
fp16-subnormal territory — so the host scales W by 4096 (exact power of two)
and divides the gathered output back.  End-to-end error vs the reference:
~1.3e-3 of output absmax (BAND=1 truncation dominates; fp16 rounding alone
is ~6e-4).
"""
import os
from contextlib import ExitStack

import numpy as np

DT = 0.15
DX = 1.0
NUM_STEPS = 4
EPS = 1e-6
S = 32
C = 3
PIX = S * S          # 1024
KC = PIX // 128      # 8 k-chunks per channel
ROW = C * PIX        # 3072 floats per batch
B_TOTAL = 16384
N_CORES = 8
B_CORE = B_TOTAL // N_CORES
BAND = int(os.environ.get("KERNEL_BAND", "1"))  # block band half-width


def _klist(h):
    """In-band k-chunks for output half h (m-chunks 4h..4h+3)."""
    return list(range(max(0, 4 * h - BAND), min(KC, 4 * h + 3 + BAND + 1)))


def _slices(h):
    """Tight column ranges per in-band k for half h: [(k, col_start, col_end)].

    Only m-chunks within BAND of k are nonzero; fp16 matmuls run 1 cyc/row at
    any width, so ranges are exactly the in-band columns.  Accumulation with
    per-k partial column ranges is safe: the start=True matmul clears the whole
    PSUM bank's has_written bits, so each element's first writer overwrites.
    """
    res = []
    for k in _klist(h):
        mlo = max(4 * h, k - BAND)
        mhi = min(4 * h + 4, k + 1 + BAND)
        res.append((k, (mlo - 4 * h) * 128, (mhi - 4 * h) * 128))
    return res


def _wtot(h):
    return sum(ce - cs for _, cs, ce in _slices(h))


_CACHE = {}
LAST_RESULTS = None  # BassKernelResults of the most recent run (for test.py)


# ----------------------------- host-side operator ---------------------------

def _smooth3(m, axis):
    p = np.concatenate([m.take([0], axis=axis), m, m.take([-1], axis=axis)],
                       axis=axis)
    n = m.shape[axis]
    sl = lambda i: p.take(range(i, i + n), axis=axis)
    return (sl(0) + sl(1) + sl(2)) / 3.0


def _thomas_matrix(a, b, c):
    """Exact linear map of the reference thomas() for one N-system, as [N,N]."""
    N = a.shape[0]
    d = np.eye(N, dtype=np.float64)
    cp = 0.0
    dp = np.zeros(N, dtype=np.float64)
    cs = np.zeros(N, dtype=np.float64)
    ds = np.zeros((N, N), dtype=np.float64)
    for i in range(N):
        denom = b[i] - a[i] * cp + EPS
        cn = c[i] / denom
        dn = (d[i] - a[i] * dp) / denom
        cs[i] = cn
        ds[i] = dn
        cp, dp = cn, dn
    cs[N - 1] = 0.0
    x = np.zeros((N, N), dtype=np.float64)
    xn = np.zeros(N, dtype=np.float64)
    for i in range(N - 1, -1, -1):
        x[i] = ds[i] - cs[i] * xn
        xn = x[i]
    return x


def _solve_matrices(coeff_smooth, dt):
    coeff = coeff_smooth * dt / (DX ** 2)
    a = -coeff
    c = -coeff
    b = 1.0 + 2.0 * coeff
    b = b.copy()
    b[..., 0] = 1.0 + coeff[..., 0]
    b[..., -1] = 1.0 + coeff[..., -1]
    Cn, K, N = a.shape
    out = np.zeros((Cn, K, N, N), dtype=np.float64)
    for ci in range(Cn):
        for k in range(K):
            out[ci, k] = _thomas_matrix(a[ci, k], b[ci, k], c[ci, k])
    return out


def _build_operator(alpha_base, beta_base, alpha_time_coeff, beta_time_coeff,
                    channel_coupling):
    """[C, 1024, 1024] float64: out_vec = L[c] @ u_vec (h*32+w order)."""
    ab = alpha_base.astype(np.float64)
    bb = beta_base.astype(np.float64)
    at = alpha_time_coeff.astype(np.float64)
    bt = beta_time_coeff.astype(np.float64)
    diag = np.diag(channel_coupling.astype(np.float64))

    M = np.broadcast_to(np.eye(PIX, dtype=np.float64).reshape(S, S, PIX),
                        (C, S, S, PIX)).copy()
    t = 0.0
    for _ in range(NUM_STEPS):
        alpha = np.maximum(ab + at * t, EPS)
        beta = np.maximum(bb + bt * t, EPS)
        Sx = _solve_matrices(_smooth3(alpha, axis=2), DT / 2)        # [C,H,w',w]
        bsm = _smooth3(beta, axis=1)
        Sy = _solve_matrices(np.transpose(bsm, (0, 2, 1)), DT)       # [C,W,h',h]
        M = np.einsum('chvw,chwK->chvK', Sx, M)
        M = np.einsum('cwuh,chwK->cuwK', Sy, M)
        M = np.einsum('chvw,chwK->chvK', Sx, M)
        M = M * diag[:, None, None, None]
        t += DT
    return M.reshape(C, PIX, PIX)


# ----------------------------- device program -------------------------------

def _build_program(nc, u_ap, w_ap, id_ap, out_ap, b_per_core):
    import concourse.tile as tile
    from concourse import mybir
    F32 = mybir.dt.float32
    F16 = mybir.dt.float16
    ntiles = b_per_core // 128
    wtot = _wtot(0)

    with tile.TileContext(nc) as tc, ExitStack() as ctx:
        const_pool = ctx.enter_context(tc.tile_pool(name="const", bufs=1))
        w_pool = ctx.enter_context(tc.tile_pool(name="w", bufs=1))
        ut_pool = ctx.enter_context(tc.tile_pool(name="ut", bufs=12))
        out_pool = ctx.enter_context(tc.tile_pool(name="out", bufs=6))
        # all 8 PSUM banks go to the matmul pipeline: 4 bufs x 2 banks.  The
        # warm-up transposes below borrow the same rotating slots.
        psm_pool = ctx.enter_context(tc.tile_pool(name="psm", bufs=4,
                                                  space="PSUM"))

        ident = const_pool.tile([128, 128], F16)
        nc.sync.dma_start(out=ident[:], in_=id_ap[:])
        # u arrives pre-transposed from the host: u_ap[tile, kk, blk*128+b]
        # (pixel-major per 128-batch tile), so each tile is ONE contiguous DMA
        # straight into the matmul operand layout - no PE transposes needed.

        # HAM warm-up: throwaway transposes of the identity bridge the PE
        # from the ident-load landing (~9us) to the first real matmuls
        # (~12us, when u0's c0 chunk and w_c0 land), so the PE never idles
        # >3.4us and the clock-gate stays at 8/8 for the first real tiles.
        for wi in range(28):
            wp = psm_pool.tile([128, 128], F16, tag="psm", name="warm")
            nc.tensor.transpose(wp[:], ident[:], ident[:])

        # All loads stream in order on the sync (SP) HWDGE ring: tile 0's u
        # arrives as three per-channel slices interleaved with the matching
        # operator slices, so channel 0's matmuls can start after ~1MB of
        # stream instead of ~3.7MB.  Stores go on the scalar (ACT) ring so a
        # store's semaphore wait never blocks a load issue (single-queue
        # FIFO was the old bottleneck).
        u_tiles = [None] * ntiles
        u_tiles[0] = ut_pool.tile([128, ROW], F16, tag="utall", name="utall")
        wt = [None] * C
        nc.sync.dma_start(out=u_tiles[0][:, 0:PIX], in_=u_ap[0][:, 0:PIX])
        for c in range(C):
            t = w_pool.tile([128, 2 * wtot], F16, tag=f"w{c}")
            nc.sync.dma_start(out=t[:], in_=w_ap[:, c * 2 * wtot:
                                                 (c + 1) * 2 * wtot])
            wt[c] = t
            if c + 1 < C:
                nc.sync.dma_start(
                    out=u_tiles[0][:, (c + 1) * PIX:(c + 2) * PIX],
                    in_=u_ap[0][:, (c + 1) * PIX:(c + 2) * PIX])

        def ut_views(utall):
            return [[utall[:, (2 * c + k // 4) * 512 + (k % 4) * 128:
                           (2 * c + k // 4) * 512 + (k % 4 + 1) * 128]
                     for k in range(KC)] for c in range(C)]

        def emit_tile(it, utall):
            ut = ut_views(utall)
            last = it == ntiles - 1
            out_nat = out_pool.tile([128, ROW], F16, name="out_nat")
            for c in range(C):
                # one 2-bank PSUM tile per channel; each 512-col half is its
                # own accumulation group confined to one bank
                ps = psm_pool.tile([128, 1024], F32, tag="psm", name="ps")
                for h in range(2):
                    sl = _slices(h)
                    off = 0
                    for i, (k, cs, ce) in enumerate(sl):
                        nc.tensor.matmul(
                            ps[:, 512 * h + cs:512 * h + ce], lhsT=ut[c][k],
                            rhs=wt[c][:, h * wtot + off:
                                      h * wtot + off + (ce - cs)],
                            start=(i == 0), stop=(i == len(sl) - 1))
                        off += ce - cs
                # psum drain: one wide copy per channel, split ACT/DVE so the
                # two PSUM read ports run in parallel (ACT also issues the
                # store, so DVE takes two of the three channels)
                if c == 0:
                    nc.scalar.copy(out_nat[:, 0:PIX], ps[:])
                else:
                    nc.vector.tensor_copy(
                        out_nat[:, c * PIX:(c + 1) * PIX], ps[:])
                if last:
                    # per-channel stores shorten the epilogue: each channel's
                    # bytes hit the wire as soon as its drain lands
                    nc.scalar.dma_start(
                        out=out_ap[it * 128:(it + 1) * 128,
                                   c * PIX:(c + 1) * PIX],
                        in_=out_nat[:, c * PIX:(c + 1) * PIX])
            if not last:
                nc.scalar.dma_start(
                    out=out_ap[it * 128:(it + 1) * 128, :], in_=out_nat[:])

        for it in range(ntiles):
            if it + 1 < ntiles:
                u_tiles[it + 1] = ut_pool.tile([128, ROW], F16, tag="utall",
                                               name="utall")
                nc.sync.dma_start(out=u_tiles[it + 1][:], in_=u_ap[it + 1])
            emit_tile(it, u_tiles[it])


def _get_nc():
    if "nc" in _CACHE:
        return _CACHE["nc"]
    from concourse import bacc, mybir
    # num_devices=1: the 8 cores are pure SPMD replicas with no collectives,
    # so skip the cross-core EVSEM butterfly in the kernel pre/postamble.
    nd = int(os.environ.get("KERNEL_ND", "1"))
    nc = bacc.Bacc("TRN2", target_bir_lowering=False, debug=False,
                   num_devices=nd)
    F16 = mybir.dt.float16
    wtot = _wtot(0)
    u_ap = nc.dram_tensor("u", [B_CORE // 128, 128, ROW], F16,
                          kind="ExternalInput").ap()
    w_ap = nc.dram_tensor("w", [128, C * 2 * wtot], F16,
                          kind="ExternalInput").ap()
    id_ap = nc.dram_tensor("ident", [128, 128], F16,
                           kind="ExternalInput").ap()
    out_ap = nc.dram_tensor("out", [B_CORE, ROW], F16,
                            kind="ExternalOutput").ap()
    _build_program(nc, u_ap, w_ap, id_ap, out_ap, B_CORE)
    nc.compile()
    _CACHE["nc"] = nc
    return nc


def _inject_ntff_hook():
    import sys, types
    try:
        import antenv.axon_hooks  # noqa: F401
        return
    except ImportError:
        pass
    from trn_agent_boot.trn_boot import _ntff_profile_via_ctypes
    hook = _ntff_profile_via_ctypes('/opt/axon/libaxon_pjrt.so')
    mod = types.ModuleType('antenv.axon_hooks')
    _state = {'hook': hook}
    mod.get_axon_ntff_profile_hook = lambda: _state['hook']
    mod.set_axon_ntff_profile_hook = lambda h: _state.update(hook=h)
    sys.modules['antenv.axon_hooks'] = mod
    import antenv
    antenv.axon_hooks = mod


# ----------------------------- entry point ----------------------------------

def kernel(u, alpha_base, beta_base, alpha_time_coeff, beta_time_coeff,
           channel_coupling):
    global LAST_RESULTS
    u = np.asarray(u, dtype=np.float32)
    assert u.shape == (B_TOTAL, C, S, S), u.shape

    L = _build_operator(np.asarray(alpha_base), np.asarray(beta_base),
                        np.asarray(alpha_time_coeff),
                        np.asarray(beta_time_coeff),
                        np.asarray(channel_coupling))
    # tight-packed banded moving-operand slices, concatenated along free dim
    wtot = _wtot(0)
    w = np.zeros((C, 2, 128, wtot), dtype=np.float32)
    LT = L.transpose(0, 2, 1).astype(np.float32)  # [c, kpix, npix]
    for h in range(2):
        off = 0
        for k, cs, ce in _slices(h):
            w[:, h, :, off:off + (ce - cs)] = \
                LT[:, k * 128:(k + 1) * 128, 512 * h + cs:512 * h + ce]
            off += ce - cs
    # x4096 (exact power of 2) lifts the ~1e-4-scale operator entries out of
    # fp16's subnormal zone; the host divides the output back
    w = (w * 4096.0).astype(np.float16)
    # partition-major relayout: w2[p, (c,h,off)] so each channel's slices are
    # one contiguous [128, 2*wtot] DMA
    w2 = np.ascontiguousarray(w.transpose(2, 0, 1, 3)).reshape(
        128, C * 2 * wtot)
    ident = np.eye(128, dtype=np.float16)

    nc = _get_nc()
    from concourse import bass_utils

    # pixel-major per 128-batch tile: u_t[tile, kk, blk*128 + b]
    u16 = u.reshape(B_TOTAL // 128, 128, ROW // 128, 128).astype(np.float16)
    u2 = np.ascontiguousarray(u16.transpose(0, 3, 2, 1)).reshape(
        B_TOTAL // 128, 128, ROW)
    tpc = B_CORE // 128
    in_maps = [{"u": u2[i * tpc:(i + 1) * tpc], "w": w2, "ident": ident}
               for i in range(N_CORES)]

    trace = os.environ.get("KERNEL_TRACE", "") == "1"
    kw = {}
    if trace:
        _inject_ntff_hook()
        bass_utils.upload_artifacts = lambda tmpdir: tmpdir
        kw = dict(trace=True, tmpdir=os.environ.get("KERNEL_TRACE_DIR"))

    # Expected result for one batch row per core, for output verification
    # (the devices occasionally fail transiently — exceptions AND, rarely,
    # silently corrupted buffers — so verify and retry).
    uf0 = u.reshape(B_TOTAL, C, PIX)
    checks = []
    for i in range(N_CORES):
        b = i * B_CORE
        checks.append(np.concatenate(
            [L[c] @ uf0[b, c].astype(np.float64) for c in range(C)]))

    import time
    last_exc = None
    for attempt in range(3):
        try:
            if trace:
                # stale ntffs from a prior run/attempt make the profiler
                # abort (parallel-instances assert) — start from a clean dir
                tdir = os.environ.get("KERNEL_TRACE_DIR")
                if tdir and os.path.isdir(tdir):
                    for f in os.listdir(tdir):
                        try:
                            os.remove(os.path.join(tdir, f))
                        except OSError:
                            pass
            res = bass_utils.run_bass_kernel_spmd(
                nc, in_maps, core_ids=list(range(N_CORES)), **kw)
        except Exception as e:
            last_exc = e
            time.sleep(5)
            continue
        ok = True
        for i in range(N_CORES):
            got = res.results[i]["out"][0].astype(np.float64) / 4096.0
            ref = checks[i]
            tol = 0.05 * max(np.abs(ref).max(), 1e-30)
            if not np.all(np.isfinite(got)) or np.abs(got - ref).max() > tol:
                ok = False
                break
        if ok:
            break
        time.sleep(5)
    else:
        if last_exc is not None:
            raise last_exc
    LAST_RESULTS = res

    out = np.concatenate([r["out"] for r in res.results], axis=0)
    out = out.astype(np.float32) * (1.0 / 4096.0)
    return out.reshape(B_TOTAL, C, S, S)


# revision 8
# speedup vs baseline: 1.0076x; 1.0052x over previous
"""Trainium2 Bass kernel for nn_CIFARDiffusionLayer (5394478923805).

The reference module is LINEAR in u:
  - every tridiagonal ADI solve has batch-independent coefficients
    (built from the tiny [C,32,32] parameter maps), and
  - einsum('cc,bchw->bchw', coupling, u) with the repeated index is a
    per-channel diagonal scale.
So the whole 4-step loop collapses, per channel, to one dense [1024,1024]
matrix L_c acting on flattened 32x32 images:  out[b,c] = L_c @ vec(u[b,c]).
L_c is built on host in float64 by pushing the 1024 basis vectors through the
exact reference recurrences (including the EPS fudge).  The operator's true
spatial support is |Δrow| <= 4 image rows, which fits exactly in a block-
TRIdiagonal structure with 128-pixel (4-row) chunks (BAND=1; rel err ~1.3e-3
of absmax vs the 2e-2 gate), so the device kernel runs a banded block
matmul — a single data-parallel pass over u (one HBM read + one write = the
memory roofline):

per 128-batch tile (per core, batch-sharded 8 ways):
  ONE contiguous DMA of the tile's pixel-major fp16 block (the host performs
  the batch<->pixel transpose while sharding - an exact relayout that removes
  all on-device transposes)
  -> fp16 banded matmuls (fp32 PSUM accumulate), data stationary / operator
     moving; per channel both 512-col halves accumulate into one 2-bank
     [128,1024] PSUM tile, drained by a single wide ACT or DVE copy
  -> one full-row [128,3072] fp16 store per tile.

Queue split (the previous all-on-sync layout serialized loads behind store
semaphore waits in the single FIFO ring and left the 16 SDMA engines ~31%
idle): u/W loads issue on the sync (SP) HWDGE ring and prefetch 8 tiles
ahead; output stores issue on the scalar (ACT) HWDGE ring right after the
PSUM drains they depend on.

Everything on-device is fp16.  The operator entries are ~1e-4 scale —
fp16-subnormal territory — so the host scales W by 4096 (exact power of two)
and divides the gathered output back.  End-to-end error vs the reference:
~1.3e-3 of output absmax (BAND=1 truncation dominates; fp16 rounding alone
is ~6e-4).
"""
import os
from contextlib import ExitStack

import numpy as np

DT = 0.15
DX = 1.0
NUM_STEPS = 4
EPS = 1e-6
S = 32
C = 3
PIX = S * S          # 1024
KC = PIX // 128      # 8 k-chunks per channel
ROW = C * PIX        # 3072 floats per batch
B_TOTAL = 16384
N_CORES = 8
B_CORE = B_TOTAL // N_CORES
BAND = int(os.environ.get("KERNEL_BAND", "1"))  # block band half-width


def _klist(h):
    """In-band k-chunks for output half h (m-chunks 4h..4h+3)."""
    return list(range(max(0, 4 * h - BAND), min(KC, 4 * h + 3 + BAND + 1)))


def _slices(h):
    """Tight column ranges per in-band k for half h: [(k, col_start, col_end)].

    Only m-chunks within BAND of k are nonzero; fp16 matmuls run 1 cyc/row at
    any width, so ranges are exactly the in-band columns.  Accumulation with
    per-k partial column ranges is safe: the start=True matmul clears the whole
    PSUM bank's has_written bits, so each element's first writer overwrites.
    """
    res = []
    for k in _klist(h):
        mlo = max(4 * h, k - BAND)
        mhi = min(4 * h + 4, k + 1 + BAND)
        res.append((k, (mlo - 4 * h) * 128, (mhi - 4 * h) * 128))
    return res


def _wtot(h):
    return sum(ce - cs for _, cs, ce in _slices(h))


_CACHE = {}
LAST_RESULTS = None  # BassKernelResults of the most recent run (for test.py)


# ----------------------------- host-side operator ---------------------------

def _smooth3(m, axis):
    p = np.concatenate([m.take([0], axis=axis), m, m.take([-1], axis=axis)],
                       axis=axis)
    n = m.shape[axis]
    sl = lambda i: p.take(range(i, i + n), axis=axis)
    return (sl(0) + sl(1) + sl(2)) / 3.0


def _thomas_matrix(a, b, c):
    """Exact linear map of the reference thomas() for one N-system, as [N,N]."""
    N = a.shape[0]
    d = np.eye(N, dtype=np.float64)
    cp = 0.0
    dp = np.zeros(N, dtype=np.float64)
    cs = np.zeros(N, dtype=np.float64)
    ds = np.zeros((N, N), dtype=np.float64)
    for i in range(N):
        denom = b[i] - a[i] * cp + EPS
        cn = c[i] / denom
        dn = (d[i] - a[i] * dp) / denom
        cs[i] = cn
        ds[i] = dn
        cp, dp = cn, dn
    cs[N - 1] = 0.0
    x = np.zeros((N, N), dtype=np.float64)
    xn = np.zeros(N, dtype=np.float64)
    for i in range(N - 1, -1, -1):
        x[i] = ds[i] - cs[i] * xn
        xn = x[i]
    return x


def _solve_matrices(coeff_smooth, dt):
    coeff = coeff_smooth * dt / (DX ** 2)
    a = -coeff
    c = -coeff
    b = 1.0 + 2.0 * coeff
    b = b.copy()
    b[..., 0] = 1.0 + coeff[..., 0]
    b[..., -1] = 1.0 + coeff[..., -1]
    Cn, K, N = a.shape
    out = np.zeros((Cn, K, N, N), dtype=np.float64)
    for ci in range(Cn):
        for k in range(K):
            out[ci, k] = _thomas_matrix(a[ci, k], b[ci, k], c[ci, k])
    return out


def _build_operator(alpha_base, beta_base, alpha_time_coeff, beta_time_coeff,
                    channel_coupling):
    """[C, 1024, 1024] float64: out_vec = L[c] @ u_vec (h*32+w order)."""
    ab = alpha_base.astype(np.float64)
    bb = beta_base.astype(np.float64)
    at = alpha_time_coeff.astype(np.float64)
    bt = beta_time_coeff.astype(np.float64)
    diag = np.diag(channel_coupling.astype(np.float64))

    M = np.broadcast_to(np.eye(PIX, dtype=np.float64).reshape(S, S, PIX),
                        (C, S, S, PIX)).copy()
    t = 0.0
    for _ in range(NUM_STEPS):
        alpha = np.maximum(ab + at * t, EPS)
        beta = np.maximum(bb + bt * t, EPS)
        Sx = _solve_matrices(_smooth3(alpha, axis=2), DT / 2)        # [C,H,w',w]
        bsm = _smooth3(beta, axis=1)
        Sy = _solve_matrices(np.transpose(bsm, (0, 2, 1)), DT)       # [C,W,h',h]
        M = np.einsum('chvw,chwK->chvK', Sx, M)
        M = np.einsum('cwuh,chwK->cuwK', Sy, M)
        M = np.einsum('chvw,chwK->chvK', Sx, M)
        M = M * diag[:, None, None, None]
        t += DT
    return M.reshape(C, PIX, PIX)


# ----------------------------- device program -------------------------------

def _build_program(nc, u_ap, w_ap, id_ap, out_ap, b_per_core):
    import concourse.tile as tile
    from concourse import mybir
    F32 = mybir.dt.float32
    F16 = mybir.dt.float16
    ntiles = b_per_core // 128
    wtot = _wtot(0)

    with tile.TileContext(nc) as tc, ExitStack() as ctx:
        const_pool = ctx.enter_context(tc.tile_pool(name="const", bufs=1))
        w_pool = ctx.enter_context(tc.tile_pool(name="w", bufs=1))
        ut_pool = ctx.enter_context(tc.tile_pool(name="ut", bufs=12))
        out_pool = ctx.enter_context(tc.tile_pool(name="out", bufs=6))
        # all 8 PSUM banks go to the matmul pipeline: 4 bufs x 2 banks.  The
        # warm-up transposes below borrow the same rotating slots.
        psm_pool = ctx.enter_context(tc.tile_pool(name="psm", bufs=4,
                                                  space="PSUM"))

        ident = const_pool.tile([128, 128], F16)
        nc.sync.dma_start(out=ident[:], in_=id_ap[:])
        # u arrives pre-transposed from the host: u_ap[tile, kk, blk*128+b]
        # (pixel-major per 128-batch tile), so each tile is ONE contiguous DMA
        # straight into the matmul operand layout - no PE transposes needed.

        # HAM warm-up: throwaway REAL matmuls (transpose-mode doesn't count
        # as PE-busy for the HAM clock-gate) bridge the PE from the
        # ident-load landing (~9us) to the first real matmuls (~12us, when
        # u0's c0 chunk and w_c0 land).  The sustained activity flips the
        # clock-gate to 8/8 before the first real tile, which otherwise
        # runs at the cold 1.2 GHz half rate for its first ~3.4us window.
        for wi in range(28):
            wp = psm_pool.tile([128, 128], F32, tag="psm", name="warm")
            nc.tensor.matmul(wp[:], lhsT=ident[:], rhs=ident[:],
                             start=True, stop=True)

        # All loads stream in order on the sync (SP) HWDGE ring: tile 0's u
        # arrives as three per-channel slices interleaved with the matching
        # operator slices, so channel 0's matmuls can start after ~1MB of
        # stream instead of ~3.7MB.  Stores go on the scalar (ACT) ring so a
        # store's semaphore wait never blocks a load issue (single-queue
        # FIFO was the old bottleneck).
        u_tiles = [None] * ntiles
        u_tiles[0] = ut_pool.tile([128, ROW], F16, tag="utall", name="utall")
        wt = [None] * C
        nc.sync.dma_start(out=u_tiles[0][:, 0:PIX], in_=u_ap[0][:, 0:PIX])
        for c in range(C):
            t = w_pool.tile([128, 2 * wtot], F16, tag=f"w{c}")
            nc.sync.dma_start(out=t[:], in_=w_ap[:, c * 2 * wtot:
                                                 (c + 1) * 2 * wtot])
            wt[c] = t
            if c + 1 < C:
                nc.sync.dma_start(
                    out=u_tiles[0][:, (c + 1) * PIX:(c + 2) * PIX],
                    in_=u_ap[0][:, (c + 1) * PIX:(c + 2) * PIX])

        def ut_views(utall):
            return [[utall[:, (2 * c + k // 4) * 512 + (k % 4) * 128:
                           (2 * c + k // 4) * 512 + (k % 4 + 1) * 128]
                     for k in range(KC)] for c in range(C)]

        def emit_tile(it, utall):
            ut = ut_views(utall)
            last = it == ntiles - 1
            out_nat = out_pool.tile([128, ROW], F16, name="out_nat")
            for c in range(C):
                # one 2-bank PSUM tile per channel; each 512-col half is its
                # own accumulation group confined to one bank
                ps = psm_pool.tile([128, 1024], F32, tag="psm", name="ps")
                for h in range(2):
                    sl = _slices(h)
                    off = 0
                    for i, (k, cs, ce) in enumerate(sl):
                        nc.tensor.matmul(
                            ps[:, 512 * h + cs:512 * h + ce], lhsT=ut[c][k],
                            rhs=wt[c][:, h * wtot + off:
                                      h * wtot + off + (ce - cs)],
                            start=(i == 0), stop=(i == len(sl) - 1))
                        off += ce - cs
                # psum drain: one wide copy per channel, split ACT/DVE so the
                # two PSUM read ports run in parallel (ACT also issues the
                # store, so DVE takes two of the three channels)
                if c == 0:
                    nc.scalar.copy(out_nat[:, 0:PIX], ps[:])
                else:
                    nc.vector.tensor_copy(
                        out_nat[:, c * PIX:(c + 1) * PIX], ps[:])
                if last:
                    # per-channel stores shorten the epilogue: each channel's
                    # bytes hit the wire as soon as its drain lands
                    nc.scalar.dma_start(
                        out=out_ap[it * 128:(it + 1) * 128,
                                   c * PIX:(c + 1) * PIX],
                        in_=out_nat[:, c * PIX:(c + 1) * PIX])
            if not last:
                nc.scalar.dma_start(
                    out=out_ap[it * 128:(it + 1) * 128, :], in_=out_nat[:])

        for it in range(ntiles):
            if it + 1 < ntiles:
                u_tiles[it + 1] = ut_pool.tile([128, ROW], F16, tag="utall",
                                               name="utall")
                nc.sync.dma_start(out=u_tiles[it + 1][:], in_=u_ap[it + 1])
            emit_tile(it, u_tiles[it])


def _get_nc():
    if "nc" in _CACHE:
        return _CACHE["nc"]
    from concourse import bacc, mybir
    # num_devices=1: the 8 cores are pure SPMD replicas with no collectives,
    # so skip the cross-core EVSEM butterfly in the kernel pre/postamble.
    nd = int(os.environ.get("KERNEL_ND", "1"))
    nc = bacc.Bacc("TRN2", target_bir_lowering=False, debug=False,
                   num_devices=nd)
    F16 = mybir.dt.float16
    wtot = _wtot(0)
    u_ap = nc.dram_tensor("u", [B_CORE // 128, 128, ROW], F16,
                          kind="ExternalInput").ap()
    w_ap = nc.dram_tensor("w", [128, C * 2 * wtot], F16,
                          kind="ExternalInput").ap()
    id_ap = nc.dram_tensor("ident", [128, 128], F16,
                           kind="ExternalInput").ap()
    out_ap = nc.dram_tensor("out", [B_CORE, ROW], F16,
                            kind="ExternalOutput").ap()
    _build_program(nc, u_ap, w_ap, id_ap, out_ap, B_CORE)
    nc.compile()
    _CACHE["nc"] = nc
    return nc


def _inject_ntff_hook():
    import sys, types
    try:
        import antenv.axon_hooks  # noqa: F401
        return
    except ImportError:
        pass
    from trn_agent_boot.trn_boot import _ntff_profile_via_ctypes
    hook = _ntff_profile_via_ctypes('/opt/axon/libaxon_pjrt.so')
    mod = types.ModuleType('antenv.axon_hooks')
    _state = {'hook': hook}
    mod.get_axon_ntff_profile_hook = lambda: _state['hook']
    mod.set_axon_ntff_profile_hook = lambda h: _state.update(hook=h)
    sys.modules['antenv.axon_hooks'] = mod
    import antenv
    antenv.axon_hooks = mod


# ----------------------------- entry point ----------------------------------

def kernel(u, alpha_base, beta_base, alpha_time_coeff, beta_time_coeff,
           channel_coupling):
    global LAST_RESULTS
    u = np.asarray(u, dtype=np.float32)
    assert u.shape == (B_TOTAL, C, S, S), u.shape

    L = _build_operator(np.asarray(alpha_base), np.asarray(beta_base),
                        np.asarray(alpha_time_coeff),
                        np.asarray(beta_time_coeff),
                        np.asarray(channel_coupling))
    # tight-packed banded moving-operand slices, concatenated along free dim
    wtot = _wtot(0)
    w = np.zeros((C, 2, 128, wtot), dtype=np.float32)
    LT = L.transpose(0, 2, 1).astype(np.float32)  # [c, kpix, npix]
    for h in range(2):
        off = 0
        for k, cs, ce in _slices(h):
            w[:, h, :, off:off + (ce - cs)] = \
                LT[:, k * 128:(k + 1) * 128, 512 * h + cs:512 * h + ce]
            off += ce - cs
    # x4096 (exact power of 2) lifts the ~1e-4-scale operator entries out of
    # fp16's subnormal zone; the host divides the output back
    w = (w * 4096.0).astype(np.float16)
    # partition-major relayout: w2[p, (c,h,off)] so each channel's slices are
    # one contiguous [128, 2*wtot] DMA
    w2 = np.ascontiguousarray(w.transpose(2, 0, 1, 3)).reshape(
        128, C * 2 * wtot)
    ident = np.eye(128, dtype=np.float16)

    nc = _get_nc()
    from concourse import bass_utils

    # pixel-major per 128-batch tile: u_t[tile, kk, blk*128 + b]
    u16 = u.reshape(B_TOTAL // 128, 128, ROW // 128, 128).astype(np.float16)
    u2 = np.ascontiguousarray(u16.transpose(0, 3, 2, 1)).reshape(
        B_TOTAL // 128, 128, ROW)
    tpc = B_CORE // 128
    in_maps = [{"u": u2[i * tpc:(i + 1) * tpc], "w": w2, "ident": ident}
               for i in range(N_CORES)]

    trace = os.environ.get("KERNEL_TRACE", "") == "1"
    kw = {}
    if trace:
        _inject_ntff_hook()
        bass_utils.upload_artifacts = lambda tmpdir: tmpdir
        kw = dict(trace=True, tmpdir=os.environ.get("KERNEL_TRACE_DIR"))

    # Expected result for one batch row per core, for output verification
    # (the devices occasionally fail transiently — exceptions AND, rarely,
    # silently corrupted buffers — so verify and retry).
    uf0 = u.reshape(B_TOTAL, C, PIX)
    checks = []
    for i in range(N_CORES):
        b = i * B_CORE
        checks.append(np.concatenate(
            [L[c] @ uf0[b, c].astype(np.float64) for c in range(C)]))

    import time
    last_exc = None
    for attempt in range(3):
        try:
            if trace:
                # stale ntffs from a prior run/attempt make the profiler
                # abort (parallel-instances assert) — start from a clean dir
                tdir = os.environ.get("KERNEL_TRACE_DIR")
                if tdir and os.path.isdir(tdir):
                    for f in os.listdir(tdir):
                        try:
                            os.remove(os.path.join(tdir, f))
                        except OSError:
                            pass
            res = bass_utils.run_bass_kernel_spmd(
                nc, in_maps, core_ids=list(range(N_CORES)), **kw)
        except Exception as e:
            last_exc = e
            time.sleep(5)
            continue
        ok = True
        for i in range(N_CORES):
            got = res.results[i]["out"][0].astype(np.float64) / 4096.0
            ref = checks[i]
            tol = 0.05 * max(np.abs(ref).max(), 1e-30)
            if not np.all(np.isfinite(got)) or np.abs(got - ref).max() > tol:
                ok = False
                break
        if ok:
            break
        time.sleep(5)
    else:
        if last_exc is not None:
            raise last_exc
    LAST_RESULTS = res

    out = np.concatenate([r["out"] for r in res.results], axis=0)
    out = out.astype(np.float32) * (1.0 / 4096.0)
    return out.reshape(B_TOTAL, C, S, S)


# revision 10
# speedup vs baseline: 1.0640x; 1.0559x over previous
"""Trainium2 Bass kernel for nn_CIFARDiffusionLayer (5394478923805).

The reference module is LINEAR in u:
  - every tridiagonal ADI solve has batch-independent coefficients
    (built from the tiny [C,32,32] parameter maps), and
  - einsum('cc,bchw->bchw', coupling, u) with the repeated index is a
    per-channel diagonal scale.
So the whole 4-step loop collapses, per channel, to one dense [1024,1024]
matrix L_c acting on flattened 32x32 images:  out[b,c] = L_c @ vec(u[b,c]).
L_c is built on host in float64 by pushing the 1024 basis vectors through the
exact reference recurrences (including the EPS fudge).  The operator's true
spatial support is |Δrow| <= 4 image rows, which fits exactly in a block-
TRIdiagonal structure with 128-pixel (4-row) chunks (BAND=1; rel err ~1.3e-3
of absmax vs the 2e-2 gate), so the device kernel runs a banded block
matmul — a single data-parallel pass over u (one HBM read + one write = the
memory roofline):

per 128-batch tile (per core, batch-sharded 8 ways):
  ONE contiguous DMA of the tile's pixel-major fp16 block (the host performs
  the batch<->pixel transpose while sharding - an exact relayout that removes
  all on-device transposes)
  -> fp16 banded matmuls (fp32 PSUM accumulate), data stationary / operator
     moving; per channel both 512-col halves accumulate into one 2-bank
     [128,1024] PSUM tile, drained by a single wide ACT or DVE copy
  -> one full-row [128,3072] fp16 store per tile.

Queue split (the previous all-on-sync layout serialized loads behind store
semaphore waits in the single FIFO ring and left the 16 SDMA engines ~31%
idle): u/W loads issue on the sync (SP) HWDGE ring and prefetch 8 tiles
ahead; output stores issue on the scalar (ACT) HWDGE ring right after the
PSUM drains they depend on.

Everything on-device is fp16.  The operator entries are ~1e-4 scale —
fp16-subnormal territory — so the host scales W by 4096 (exact power of two)
and divides the gathered output back.  End-to-end error vs the reference:
~1.3e-3 of output absmax (BAND=1 truncation dominates; fp16 rounding alone
is ~6e-4).
"""
import os
from contextlib import ExitStack

import numpy as np

DT = 0.15
DX = 1.0
NUM_STEPS = 4
EPS = 1e-6
S = 32
C = 3
PIX = S * S          # 1024
KC = PIX // 128      # 8 k-chunks per channel
ROW = C * PIX        # 3072 floats per batch
B_TOTAL = 16384
N_CORES = 8
B_CORE = B_TOTAL // N_CORES
BAND = int(os.environ.get("KERNEL_BAND", "1"))  # block band half-width


def _klist(h):
    """In-band k-chunks for output half h (m-chunks 4h..4h+3)."""
    return list(range(max(0, 4 * h - BAND), min(KC, 4 * h + 3 + BAND + 1)))


def _slices(h):
    """Tight column ranges per in-band k for half h: [(k, col_start, col_end)].

    Only m-chunks within BAND of k are nonzero; fp16 matmuls run 1 cyc/row at
    any width, so ranges are exactly the in-band columns.  Accumulation with
    per-k partial column ranges is safe: the start=True matmul clears the whole
    PSUM bank's has_written bits, so each element's first writer overwrites.
    """
    res = []
    for k in _klist(h):
        mlo = max(4 * h, k - BAND)
        mhi = min(4 * h + 4, k + 1 + BAND)
        res.append((k, (mlo - 4 * h) * 128, (mhi - 4 * h) * 128))
    return res


def _wtot(h):
    return sum(ce - cs for _, cs, ce in _slices(h))


_CACHE = {}
LAST_RESULTS = None  # BassKernelResults of the most recent run (for test.py)


# ----------------------------- host-side operator ---------------------------

def _smooth3(m, axis):
    p = np.concatenate([m.take([0], axis=axis), m, m.take([-1], axis=axis)],
                       axis=axis)
    n = m.shape[axis]
    sl = lambda i: p.take(range(i, i + n), axis=axis)
    return (sl(0) + sl(1) + sl(2)) / 3.0


def _thomas_matrix(a, b, c):
    """Exact linear map of the reference thomas() for one N-system, as [N,N]."""
    N = a.shape[0]
    d = np.eye(N, dtype=np.float64)
    cp = 0.0
    dp = np.zeros(N, dtype=np.float64)
    cs = np.zeros(N, dtype=np.float64)
    ds = np.zeros((N, N), dtype=np.float64)
    for i in range(N):
        denom = b[i] - a[i] * cp + EPS
        cn = c[i] / denom
        dn = (d[i] - a[i] * dp) / denom
        cs[i] = cn
        ds[i] = dn
        cp, dp = cn, dn
    cs[N - 1] = 0.0
    x = np.zeros((N, N), dtype=np.float64)
    xn = np.zeros(N, dtype=np.float64)
    for i in range(N - 1, -1, -1):
        x[i] = ds[i] - cs[i] * xn
        xn = x[i]
    return x


def _solve_matrices(coeff_smooth, dt):
    coeff = coeff_smooth * dt / (DX ** 2)
    a = -coeff
    c = -coeff
    b = 1.0 + 2.0 * coeff
    b = b.copy()
    b[..., 0] = 1.0 + coeff[..., 0]
    b[..., -1] = 1.0 + coeff[..., -1]
    Cn, K, N = a.shape
    out = np.zeros((Cn, K, N, N), dtype=np.float64)
    for ci in range(Cn):
        for k in range(K):
            out[ci, k] = _thomas_matrix(a[ci, k], b[ci, k], c[ci, k])
    return out


def _build_operator(alpha_base, beta_base, alpha_time_coeff, beta_time_coeff,
                    channel_coupling):
    """[C, 1024, 1024] float64: out_vec = L[c] @ u_vec (h*32+w order)."""
    ab = alpha_base.astype(np.float64)
    bb = beta_base.astype(np.float64)
    at = alpha_time_coeff.astype(np.float64)
    bt = beta_time_coeff.astype(np.float64)
    diag = np.diag(channel_coupling.astype(np.float64))

    M = np.broadcast_to(np.eye(PIX, dtype=np.float64).reshape(S, S, PIX),
                        (C, S, S, PIX)).copy()
    t = 0.0
    for _ in range(NUM_STEPS):
        alpha = np.maximum(ab + at * t, EPS)
        beta = np.maximum(bb + bt * t, EPS)
        Sx = _solve_matrices(_smooth3(alpha, axis=2), DT / 2)        # [C,H,w',w]
        bsm = _smooth3(beta, axis=1)
        Sy = _solve_matrices(np.transpose(bsm, (0, 2, 1)), DT)       # [C,W,h',h]
        M = np.einsum('chvw,chwK->chvK', Sx, M)
        M = np.einsum('cwuh,chwK->cuwK', Sy, M)
        M = np.einsum('chvw,chwK->chvK', Sx, M)
        M = M * diag[:, None, None, None]
        t += DT
    return M.reshape(C, PIX, PIX)


# ----------------------------- device program -------------------------------

def _build_program(nc, u_ap, w_ap, id_ap, out_ap, b_per_core):
    import concourse.tile as tile
    from concourse import mybir
    F32 = mybir.dt.float32
    F16 = mybir.dt.float16
    ntiles = b_per_core // 128
    wtot = _wtot(0)

    with tile.TileContext(nc) as tc, ExitStack() as ctx:
        const_pool = ctx.enter_context(tc.tile_pool(name="const", bufs=1))
        w_pool = ctx.enter_context(tc.tile_pool(name="w", bufs=1))
        ut_pool = ctx.enter_context(tc.tile_pool(name="ut", bufs=12))
        out_pool = ctx.enter_context(tc.tile_pool(name="out", bufs=6))
        # all 8 PSUM banks go to the matmul pipeline: 4 bufs x 2 banks.  The
        # warm-up transposes below borrow the same rotating slots.
        psm_pool = ctx.enter_context(tc.tile_pool(name="psm", bufs=4,
                                                  space="PSUM"))

        ident = const_pool.tile([128, 128], F16)
        nc.sync.dma_start(out=ident[:], in_=id_ap[:])
        # u arrives pre-transposed from the host: u_ap[tile, kk, blk*128+b]
        # (pixel-major per 128-batch tile), so each tile is ONE contiguous DMA
        # straight into the matmul operand layout - no PE transposes needed.

        # HAM warm-up: throwaway REAL matmuls (transpose-mode doesn't count
        # as PE-busy for the HAM clock-gate) bridge the PE from the
        # ident-load landing (~9us) to the first real matmuls (~12us, when
        # u0's c0 chunk and w_c0 land).  The sustained activity flips the
        # clock-gate to 8/8 before the first real tile, which otherwise
        # runs at the cold 1.2 GHz half rate for its first ~3.4us window.
        for wi in range(28):
            wp = psm_pool.tile([128, 128], F32, tag="psm", name="warm")
            nc.tensor.matmul(wp[:], lhsT=ident[:], rhs=ident[:],
                             start=True, stop=True)

        # All loads stream in order on the sync (SP) HWDGE ring: tile 0's u
        # arrives as three per-channel slices interleaved with the matching
        # operator slices, so channel 0's matmuls can start after ~1MB of
        # stream instead of ~3.7MB.  Stores go on the scalar (ACT) ring so a
        # store's semaphore wait never blocks a load issue (single-queue
        # FIFO was the old bottleneck).
        u_tiles = [None] * ntiles
        u_tiles[0] = ut_pool.tile([128, ROW], F16, tag="utall", name="utall")
        wt = [None] * C
        nc.sync.dma_start(out=u_tiles[0][:, 0:PIX], in_=u_ap[0][:, 0:PIX])
        for c in range(C):
            t = w_pool.tile([128, 2 * wtot], F16, tag=f"w{c}")
            if c == 0:
                # half-granular first operator load: channel 0's h0 matmuls
                # only need u0c0 + w0h0 (~0.9MB of stream)
                nc.sync.dma_start(out=t[:, 0:wtot], in_=w_ap[:, 0:wtot])
                nc.sync.dma_start(out=t[:, wtot:2 * wtot],
                                  in_=w_ap[:, wtot:2 * wtot])
            else:
                nc.sync.dma_start(out=t[:], in_=w_ap[:, c * 2 * wtot:
                                                     (c + 1) * 2 * wtot])
            wt[c] = t
            if c + 1 < C:
                nc.sync.dma_start(
                    out=u_tiles[0][:, (c + 1) * PIX:(c + 2) * PIX],
                    in_=u_ap[0][:, (c + 1) * PIX:(c + 2) * PIX])

        def ut_views(utall):
            return [[utall[:, (2 * c + k // 4) * 512 + (k % 4) * 128:
                           (2 * c + k // 4) * 512 + (k % 4 + 1) * 128]
                     for k in range(KC)] for c in range(C)]

        def emit_tile(it, utall):
            ut = ut_views(utall)
            last = it == ntiles - 1
            out_nat = out_pool.tile([128, ROW], F16, name="out_nat")
            for c in range(C):
                # one 2-bank PSUM tile per channel; each 512-col half is its
                # own accumulation group confined to one bank
                ps = psm_pool.tile([128, 1024], F32, tag="psm", name="ps")
                for h in range(2):
                    sl = _slices(h)
                    off = 0
                    for i, (k, cs, ce) in enumerate(sl):
                        nc.tensor.matmul(
                            ps[:, 512 * h + cs:512 * h + ce], lhsT=ut[c][k],
                            rhs=wt[c][:, h * wtot + off:
                                      h * wtot + off + (ce - cs)],
                            start=(i == 0), stop=(i == len(sl) - 1))
                        off += ce - cs
                # psum drain: one wide copy per channel, split ACT/DVE so the
                # two PSUM read ports run in parallel (ACT also issues the
                # store, so DVE takes two of the three channels).  The last
                # tile drains at half-channel granularity so the final
                # copy->store->HBM-receipt chain is as short as possible.
                if last:
                    for hh in range(2):
                        lo = c * PIX + hh * 512
                        if c == 0:
                            nc.scalar.copy(out_nat[:, lo:lo + 512],
                                           ps[:, hh * 512:(hh + 1) * 512])
                        else:
                            nc.vector.tensor_copy(
                                out_nat[:, lo:lo + 512],
                                ps[:, hh * 512:(hh + 1) * 512])
                        nc.scalar.dma_start(
                            out=out_ap[it * 128:(it + 1) * 128, lo:lo + 512],
                            in_=out_nat[:, lo:lo + 512])
                elif c == 0:
                    nc.scalar.copy(out_nat[:, 0:PIX], ps[:])
                else:
                    nc.vector.tensor_copy(
                        out_nat[:, c * PIX:(c + 1) * PIX], ps[:])
            if not last:
                nc.scalar.dma_start(
                    out=out_ap[it * 128:(it + 1) * 128, :], in_=out_nat[:])

        for it in range(ntiles):
            if it + 1 < ntiles:
                u_tiles[it + 1] = ut_pool.tile([128, ROW], F16, tag="utall",
                                               name="utall")
                nc.sync.dma_start(out=u_tiles[it + 1][:], in_=u_ap[it + 1])
            emit_tile(it, u_tiles[it])


def _get_nc():
    if "nc" in _CACHE:
        return _CACHE["nc"]
    from concourse import bacc, mybir
    # num_devices=1: the 8 cores are pure SPMD replicas with no collectives,
    # so skip the cross-core EVSEM butterfly in the kernel pre/postamble.
    nd = int(os.environ.get("KERNEL_ND", "1"))
    nc = bacc.Bacc("TRN2", target_bir_lowering=False, debug=False,
                   num_devices=nd)
    F16 = mybir.dt.float16
    wtot = _wtot(0)
    u_ap = nc.dram_tensor("u", [B_CORE // 128, 128, ROW], F16,
                          kind="ExternalInput").ap()
    w_ap = nc.dram_tensor("w", [128, C * 2 * wtot], F16,
                          kind="ExternalInput").ap()
    id_ap = nc.dram_tensor("ident", [128, 128], F16,
                           kind="ExternalInput").ap()
    out_ap = nc.dram_tensor("out", [B_CORE, ROW], F16,
                            kind="ExternalOutput").ap()
    _build_program(nc, u_ap, w_ap, id_ap, out_ap, B_CORE)
    nc.compile()
    _CACHE["nc"] = nc
    return nc


def _inject_ntff_hook():
    import sys, types
    try:
        import antenv.axon_hooks  # noqa: F401
        return
    except ImportError:
        pass
    from trn_agent_boot.trn_boot import _ntff_profile_via_ctypes
    hook = _ntff_profile_via_ctypes('/opt/axon/libaxon_pjrt.so')
    mod = types.ModuleType('antenv.axon_hooks')
    _state = {'hook': hook}
    mod.get_axon_ntff_profile_hook = lambda: _state['hook']
    mod.set_axon_ntff_profile_hook = lambda h: _state.update(hook=h)
    sys.modules['antenv.axon_hooks'] = mod
    import antenv
    antenv.axon_hooks = mod


# ----------------------------- entry point ----------------------------------

def kernel(u, alpha_base, beta_base, alpha_time_coeff, beta_time_coeff,
           channel_coupling):
    global LAST_RESULTS
    u = np.asarray(u, dtype=np.float32)
    assert u.shape == (B_TOTAL, C, S, S), u.shape

    L = _build_operator(np.asarray(alpha_base), np.asarray(beta_base),
                        np.asarray(alpha_time_coeff),
                        np.asarray(beta_time_coeff),
                        np.asarray(channel_coupling))
    # tight-packed banded moving-operand slices, concatenated along free dim
    wtot = _wtot(0)
    w = np.zeros((C, 2, 128, wtot), dtype=np.float32)
    LT = L.transpose(0, 2, 1).astype(np.float32)  # [c, kpix, npix]
    for h in range(2):
        off = 0
        for k, cs, ce in _slices(h):
            w[:, h, :, off:off + (ce - cs)] = \
                LT[:, k * 128:(k + 1) * 128, 512 * h + cs:512 * h + ce]
            off += ce - cs
    # x4096 (exact power of 2) lifts the ~1e-4-scale operator entries out of
    # fp16's subnormal zone; the host divides the output back
    w = (w * 4096.0).astype(np.float16)
    # partition-major relayout: w2[p, (c,h,off)] so each channel's slices are
    # one contiguous [128, 2*wtot] DMA
    w2 = np.ascontiguousarray(w.transpose(2, 0, 1, 3)).reshape(
        128, C * 2 * wtot)
    ident = np.eye(128, dtype=np.float16)

    nc = _get_nc()
    from concourse import bass_utils

    # pixel-major per 128-batch tile: u_t[tile, kk, blk*128 + b]
    u16 = u.reshape(B_TOTAL // 128, 128, ROW // 128, 128).astype(np.float16)
    u2 = np.ascontiguousarray(u16.transpose(0, 3, 2, 1)).reshape(
        B_TOTAL // 128, 128, ROW)
    tpc = B_CORE // 128
    in_maps = [{"u": u2[i * tpc:(i + 1) * tpc], "w": w2, "ident": ident}
               for i in range(N_CORES)]

    trace = os.environ.get("KERNEL_TRACE", "") == "1"
    kw = {}
    if trace:
        _inject_ntff_hook()
        bass_utils.upload_artifacts = lambda tmpdir: tmpdir
        kw = dict(trace=True, tmpdir=os.environ.get("KERNEL_TRACE_DIR"))

    # Expected result for one batch row per core, for output verification
    # (the devices occasionally fail transiently — exceptions AND, rarely,
    # silently corrupted buffers — so verify and retry).
    uf0 = u.reshape(B_TOTAL, C, PIX)
    checks = []
    for i in range(N_CORES):
        b = i * B_CORE
        checks.append(np.concatenate(
            [L[c] @ uf0[b, c].astype(np.float64) for c in range(C)]))

    import time
    last_exc = None
    for attempt in range(3):
        try:
            if trace:
                # stale ntffs from a prior run/attempt make the profiler
                # abort (parallel-instances assert) — start from a clean dir
                tdir = os.environ.get("KERNEL_TRACE_DIR")
                if tdir and os.path.isdir(tdir):
                    for f in os.listdir(tdir):
                        try:
                            os.remove(os.path.join(tdir, f))
                        except OSError:
                            pass
            res = bass_utils.run_bass_kernel_spmd(
                nc, in_maps, core_ids=list(range(N_CORES)), **kw)
        except Exception as e:
            last_exc = e
            time.sleep(5)
            continue
        ok = True
        for i in range(N_CORES):
            got = res.results[i]["out"][0].astype(np.float64) / 4096.0
            ref = checks[i]
            tol = 0.05 * max(np.abs(ref).max(), 1e-30)
            if not np.all(np.isfinite(got)) or np.abs(got - ref).max() > tol:
                ok = False
                break
        if ok:
            break
        time.sleep(5)
    else:
        if last_exc is not None:
            raise last_exc
    LAST_RESULTS = res

    out = np.concatenate([r["out"] for r in res.results], axis=0)
    out = out.astype(np.float32) * (1.0 / 4096.0)
    return out.reshape(B_TOTAL, C, S, S)


# revision 12
# speedup vs baseline: 1.0655x; 1.0015x over previous
"""Trainium2 Bass kernel for nn_CIFARDiffusionLayer (5394478923805).

The reference module is LINEAR in u:
  - every tridiagonal ADI solve has batch-independent coefficients
    (built from the tiny [C,32,32] parameter maps), and
  - einsum('cc,bchw->bchw', coupling, u) with the repeated index is a
    per-channel diagonal scale.
So the whole 4-step loop collapses, per channel, to one dense [1024,1024]
matrix L_c acting on flattened 32x32 images:  out[b,c] = L_c @ vec(u[b,c]).
L_c is built on host in float64 by pushing the 1024 basis vectors through the
exact reference recurrences (including the EPS fudge).  The operator's true
spatial support is |Δrow| <= 4 image rows, which fits exactly in a block-
TRIdiagonal structure with 128-pixel (4-row) chunks (BAND=1; rel err ~1.3e-3
of absmax vs the 2e-2 gate), so the device kernel runs a banded block
matmul — a single data-parallel pass over u (one HBM read + one write = the
memory roofline):

per 128-batch tile (per core, batch-sharded 8 ways):
  ONE contiguous DMA of the tile's pixel-major fp16 block (the host performs
  the batch<->pixel transpose while sharding - an exact relayout that removes
  all on-device transposes)
  -> fp16 banded matmuls (fp32 PSUM accumulate), data stationary / operator
     moving; per channel both 512-col halves accumulate into one 2-bank
     [128,1024] PSUM tile, drained by a single wide ACT or DVE copy
  -> one full-row [128,3072] fp16 store per tile.

Queue split (the previous all-on-sync layout serialized loads behind store
semaphore waits in the single FIFO ring and left the 16 SDMA engines ~31%
idle): u/W loads issue on the sync (SP) HWDGE ring and prefetch 8 tiles
ahead; output stores issue on the scalar (ACT) HWDGE ring right after the
PSUM drains they depend on.

Everything on-device is fp16.  The operator entries are ~1e-4 scale —
fp16-subnormal territory — so the host scales W by 4096 (exact power of two)
and divides the gathered output back.  End-to-end error vs the reference:
~1.3e-3 of output absmax (BAND=1 truncation dominates; fp16 rounding alone
is ~6e-4).
"""
import os
from contextlib import ExitStack

import numpy as np

DT = 0.15
DX = 1.0
NUM_STEPS = 4
EPS = 1e-6
S = 32
C = 3
PIX = S * S          # 1024
KC = PIX // 128      # 8 k-chunks per channel
ROW = C * PIX        # 3072 floats per batch
B_TOTAL = 16384
N_CORES = 8
B_CORE = B_TOTAL // N_CORES
BAND = int(os.environ.get("KERNEL_BAND", "1"))  # block band half-width


def _klist(h):
    """In-band k-chunks for output half h (m-chunks 4h..4h+3)."""
    return list(range(max(0, 4 * h - BAND), min(KC, 4 * h + 3 + BAND + 1)))


def _slices(h):
    """Tight column ranges per in-band k for half h: [(k, col_start, col_end)].

    Only m-chunks within BAND of k are nonzero; fp16 matmuls run 1 cyc/row at
    any width, so ranges are exactly the in-band columns.  Accumulation with
    per-k partial column ranges is safe: the start=True matmul clears the whole
    PSUM bank's has_written bits, so each element's first writer overwrites.
    """
    res = []
    for k in _klist(h):
        mlo = max(4 * h, k - BAND)
        mhi = min(4 * h + 4, k + 1 + BAND)
        res.append((k, (mlo - 4 * h) * 128, (mhi - 4 * h) * 128))
    return res


def _wtot(h):
    return sum(ce - cs for _, cs, ce in _slices(h))


_CACHE = {}
LAST_RESULTS = None  # BassKernelResults of the most recent run (for test.py)


# ----------------------------- host-side operator ---------------------------

def _smooth3(m, axis):
    p = np.concatenate([m.take([0], axis=axis), m, m.take([-1], axis=axis)],
                       axis=axis)
    n = m.shape[axis]
    sl = lambda i: p.take(range(i, i + n), axis=axis)
    return (sl(0) + sl(1) + sl(2)) / 3.0


def _thomas_matrix(a, b, c):
    """Exact linear map of the reference thomas() for one N-system, as [N,N]."""
    N = a.shape[0]
    d = np.eye(N, dtype=np.float64)
    cp = 0.0
    dp = np.zeros(N, dtype=np.float64)
    cs = np.zeros(N, dtype=np.float64)
    ds = np.zeros((N, N), dtype=np.float64)
    for i in range(N):
        denom = b[i] - a[i] * cp + EPS
        cn = c[i] / denom
        dn = (d[i] - a[i] * dp) / denom
        cs[i] = cn
        ds[i] = dn
        cp, dp = cn, dn
    cs[N - 1] = 0.0
    x = np.zeros((N, N), dtype=np.float64)
    xn = np.zeros(N, dtype=np.float64)
    for i in range(N - 1, -1, -1):
        x[i] = ds[i] - cs[i] * xn
        xn = x[i]
    return x


def _solve_matrices(coeff_smooth, dt):
    coeff = coeff_smooth * dt / (DX ** 2)
    a = -coeff
    c = -coeff
    b = 1.0 + 2.0 * coeff
    b = b.copy()
    b[..., 0] = 1.0 + coeff[..., 0]
    b[..., -1] = 1.0 + coeff[..., -1]
    Cn, K, N = a.shape
    out = np.zeros((Cn, K, N, N), dtype=np.float64)
    for ci in range(Cn):
        for k in range(K):
            out[ci, k] = _thomas_matrix(a[ci, k], b[ci, k], c[ci, k])
    return out


def _build_operator(alpha_base, beta_base, alpha_time_coeff, beta_time_coeff,
                    channel_coupling):
    """[C, 1024, 1024] float64: out_vec = L[c] @ u_vec (h*32+w order)."""
    ab = alpha_base.astype(np.float64)
    bb = beta_base.astype(np.float64)
    at = alpha_time_coeff.astype(np.float64)
    bt = beta_time_coeff.astype(np.float64)
    diag = np.diag(channel_coupling.astype(np.float64))

    M = np.broadcast_to(np.eye(PIX, dtype=np.float64).reshape(S, S, PIX),
                        (C, S, S, PIX)).copy()
    t = 0.0
    for _ in range(NUM_STEPS):
        alpha = np.maximum(ab + at * t, EPS)
        beta = np.maximum(bb + bt * t, EPS)
        Sx = _solve_matrices(_smooth3(alpha, axis=2), DT / 2)        # [C,H,w',w]
        bsm = _smooth3(beta, axis=1)
        Sy = _solve_matrices(np.transpose(bsm, (0, 2, 1)), DT)       # [C,W,h',h]
        M = np.einsum('chvw,chwK->chvK', Sx, M)
        M = np.einsum('cwuh,chwK->cuwK', Sy, M)
        M = np.einsum('chvw,chwK->chvK', Sx, M)
        M = M * diag[:, None, None, None]
        t += DT
    return M.reshape(C, PIX, PIX)


# ----------------------------- device program -------------------------------

def _build_program(nc, u_ap, w_ap, id_ap, out_ap, b_per_core):
    import concourse.tile as tile
    from concourse import mybir
    F32 = mybir.dt.float32
    F16 = mybir.dt.float16
    ntiles = b_per_core // 128
    wtot = _wtot(0)

    with tile.TileContext(nc) as tc, ExitStack() as ctx:
        const_pool = ctx.enter_context(tc.tile_pool(name="const", bufs=1))
        w_pool = ctx.enter_context(tc.tile_pool(name="w", bufs=1))
        ut_pool = ctx.enter_context(tc.tile_pool(name="ut", bufs=12))
        out_pool = ctx.enter_context(tc.tile_pool(name="out", bufs=6))
        # all 8 PSUM banks go to the matmul pipeline: 4 bufs x 2 banks.  The
        # warm-up transposes below borrow the same rotating slots.
        psm_pool = ctx.enter_context(tc.tile_pool(name="psm", bufs=4,
                                                  space="PSUM"))

        ident = const_pool.tile([128, 128], F16)
        nc.sync.dma_start(out=ident[:], in_=id_ap[:])
        # u arrives pre-transposed from the host: u_ap[tile, kk, blk*128+b]
        # (pixel-major per 128-batch tile), so each tile is ONE contiguous DMA
        # straight into the matmul operand layout - no PE transposes needed.

        # HAM warm-up: throwaway REAL matmuls (transpose-mode doesn't count
        # as PE-busy for the HAM clock-gate) bridge the PE from the
        # ident-load landing (~9us) to the first real matmuls (~13us, when
        # u0's c0 chunk and w_c0h0 land).  The sustained activity flips the
        # clock-gate to 8/8 before the first real tile, which otherwise
        # runs at the cold 1.2 GHz half rate for its first ~3.4us window.
        for wi in range(22):
            wp = psm_pool.tile([128, 128], F32, tag="psm", name="warm")
            nc.tensor.matmul(wp[:], lhsT=ident[:], rhs=ident[:],
                             start=True, stop=True)

        # Prologue loads feed BOTH HWDGE rings in parallel: tile 0's u
        # slices on the sync (SP) ring, the operator slices on the scalar
        # (ACT) ring (idle until the first PSUM drain ~6us later), halving
        # the cold-HBM ramp before channel 0's first matmuls.  Steady-state
        # u loads stay on sync and stores on scalar, so a store's semaphore
        # wait never blocks a load issue (single-queue FIFO was the old
        # bottleneck).
        u_tiles = [None] * ntiles
        u_tiles[0] = ut_pool.tile([128, ROW], F16, tag="utall", name="utall")
        wt = [None] * C
        for c in range(C):
            nc.sync.dma_start(out=u_tiles[0][:, c * PIX:(c + 1) * PIX],
                              in_=u_ap[0][:, c * PIX:(c + 1) * PIX])
        for c in range(C):
            t = w_pool.tile([128, 2 * wtot], F16, tag=f"w{c}")
            if c == 0:
                # half-granular first operator load: channel 0's h0 matmuls
                # only need u0c0 + w0h0 (~0.9MB of stream)
                nc.scalar.dma_start(out=t[:, 0:wtot], in_=w_ap[:, 0:wtot])
                nc.scalar.dma_start(out=t[:, wtot:2 * wtot],
                                    in_=w_ap[:, wtot:2 * wtot])
            else:
                nc.scalar.dma_start(out=t[:], in_=w_ap[:, c * 2 * wtot:
                                                       (c + 1) * 2 * wtot])
            wt[c] = t

        def ut_views(utall):
            return [[utall[:, (2 * c + k // 4) * 512 + (k % 4) * 128:
                           (2 * c + k // 4) * 512 + (k % 4 + 1) * 128]
                     for k in range(KC)] for c in range(C)]

        def emit_tile(it, utall):
            ut = ut_views(utall)
            last = it == ntiles - 1
            out_nat = out_pool.tile([128, ROW], F16, name="out_nat")
            for c in range(C):
                # one 2-bank PSUM tile per channel; each 512-col half is its
                # own accumulation group confined to one bank
                ps = psm_pool.tile([128, 1024], F32, tag="psm", name="ps")
                for h in range(2):
                    sl = _slices(h)
                    off = 0
                    for i, (k, cs, ce) in enumerate(sl):
                        nc.tensor.matmul(
                            ps[:, 512 * h + cs:512 * h + ce], lhsT=ut[c][k],
                            rhs=wt[c][:, h * wtot + off:
                                      h * wtot + off + (ce - cs)],
                            start=(i == 0), stop=(i == len(sl) - 1))
                        off += ce - cs
                # psum drain: one wide copy per channel, split ACT/DVE so the
                # two PSUM read ports run in parallel (ACT also issues the
                # store, so DVE takes two of the three channels).  The last
                # tile drains at half-channel granularity so the final
                # copy->store->HBM-receipt chain is as short as possible.
                if last:
                    for hh in range(2):
                        lo = c * PIX + hh * 512
                        if c == 0:
                            nc.scalar.copy(out_nat[:, lo:lo + 512],
                                           ps[:, hh * 512:(hh + 1) * 512])
                        else:
                            nc.vector.tensor_copy(
                                out_nat[:, lo:lo + 512],
                                ps[:, hh * 512:(hh + 1) * 512])
                        nc.scalar.dma_start(
                            out=out_ap[it * 128:(it + 1) * 128, lo:lo + 512],
                            in_=out_nat[:, lo:lo + 512])
                elif c == 0:
                    nc.scalar.copy(out_nat[:, 0:PIX], ps[:])
                else:
                    nc.vector.tensor_copy(
                        out_nat[:, c * PIX:(c + 1) * PIX], ps[:])
            if not last:
                nc.scalar.dma_start(
                    out=out_ap[it * 128:(it + 1) * 128, :], in_=out_nat[:])

        for it in range(ntiles):
            if it + 1 < ntiles:
                u_tiles[it + 1] = ut_pool.tile([128, ROW], F16, tag="utall",
                                               name="utall")
                nc.sync.dma_start(out=u_tiles[it + 1][:], in_=u_ap[it + 1])
            emit_tile(it, u_tiles[it])


def _get_nc():
    if "nc" in _CACHE:
        return _CACHE["nc"]
    from concourse import bacc, mybir
    # num_devices=1: the 8 cores are pure SPMD replicas with no collectives,
    # so skip the cross-core EVSEM butterfly in the kernel pre/postamble.
    nd = int(os.environ.get("KERNEL_ND", "1"))
    nc = bacc.Bacc("TRN2", target_bir_lowering=False, debug=False,
                   num_devices=nd)
    F16 = mybir.dt.float16
    wtot = _wtot(0)
    u_ap = nc.dram_tensor("u", [B_CORE // 128, 128, ROW], F16,
                          kind="ExternalInput").ap()
    w_ap = nc.dram_tensor("w", [128, C * 2 * wtot], F16,
                          kind="ExternalInput").ap()
    id_ap = nc.dram_tensor("ident", [128, 128], F16,
                           kind="ExternalInput").ap()
    out_ap = nc.dram_tensor("out", [B_CORE, ROW], F16,
                            kind="ExternalOutput").ap()
    _build_program(nc, u_ap, w_ap, id_ap, out_ap, B_CORE)
    nc.compile()
    _CACHE["nc"] = nc
    return nc


def _inject_ntff_hook():
    import sys, types
    try:
        import antenv.axon_hooks  # noqa: F401
        return
    except ImportError:
        pass
    from trn_agent_boot.trn_boot import _ntff_profile_via_ctypes
    hook = _ntff_profile_via_ctypes('/opt/axon/libaxon_pjrt.so')
    mod = types.ModuleType('antenv.axon_hooks')
    _state = {'hook': hook}
    mod.get_axon_ntff_profile_hook = lambda: _state['hook']
    mod.set_axon_ntff_profile_hook = lambda h: _state.update(hook=h)
    sys.modules['antenv.axon_hooks'] = mod
    import antenv
    antenv.axon_hooks = mod


# ----------------------------- entry point ----------------------------------

def kernel(u, alpha_base, beta_base, alpha_time_coeff, beta_time_coeff,
           channel_coupling):
    global LAST_RESULTS
    u = np.asarray(u, dtype=np.float32)
    assert u.shape == (B_TOTAL, C, S, S), u.shape

    L = _build_operator(np.asarray(alpha_base), np.asarray(beta_base),
                        np.asarray(alpha_time_coeff),
                        np.asarray(beta_time_coeff),
                        np.asarray(channel_coupling))
    # tight-packed banded moving-operand slices, concatenated along free dim
    wtot = _wtot(0)
    w = np.zeros((C, 2, 128, wtot), dtype=np.float32)
    LT = L.transpose(0, 2, 1).astype(np.float32)  # [c, kpix, npix]
    for h in range(2):
        off = 0
        for k, cs, ce in _slices(h):
            w[:, h, :, off:off + (ce - cs)] = \
                LT[:, k * 128:(k + 1) * 128, 512 * h + cs:512 * h + ce]
            off += ce - cs
    # x4096 (exact power of 2) lifts the ~1e-4-scale operator entries out of
    # fp16's subnormal zone; the host divides the output back
    w = (w * 4096.0).astype(np.float16)
    # partition-major relayout: w2[p, (c,h,off)] so each channel's slices are
    # one contiguous [128, 2*wtot] DMA
    w2 = np.ascontiguousarray(w.transpose(2, 0, 1, 3)).reshape(
        128, C * 2 * wtot)
    ident = np.eye(128, dtype=np.float16)

    nc = _get_nc()
    from concourse import bass_utils

    # pixel-major per 128-batch tile: u_t[tile, kk, blk*128 + b]
    u16 = u.reshape(B_TOTAL // 128, 128, ROW // 128, 128).astype(np.float16)
    u2 = np.ascontiguousarray(u16.transpose(0, 3, 2, 1)).reshape(
        B_TOTAL // 128, 128, ROW)
    tpc = B_CORE // 128
    in_maps = [{"u": u2[i * tpc:(i + 1) * tpc], "w": w2, "ident": ident}
               for i in range(N_CORES)]

    trace = os.environ.get("KERNEL_TRACE", "") == "1"
    kw = {}
    if trace:
        _inject_ntff_hook()
        bass_utils.upload_artifacts = lambda tmpdir: tmpdir
        kw = dict(trace=True, tmpdir=os.environ.get("KERNEL_TRACE_DIR"))

    # Expected result for one batch row per core, for output verification
    # (the devices occasionally fail transiently — exceptions AND, rarely,
    # silently corrupted buffers — so verify and retry).
    uf0 = u.reshape(B_TOTAL, C, PIX)
    checks = []
    for i in range(N_CORES):
        b = i * B_CORE
        checks.append(np.concatenate(
            [L[c] @ uf0[b, c].astype(np.float64) for c in range(C)]))

    import time
    last_exc = None
    for attempt in range(3):
        try:
            if trace:
                # stale ntffs from a prior run/attempt make the profiler
                # abort (parallel-instances assert) — start from a clean dir
                tdir = os.environ.get("KERNEL_TRACE_DIR")
                if tdir and os.path.isdir(tdir):
                    for f in os.listdir(tdir):
                        try:
                            os.remove(os.path.join(tdir, f))
                        except OSError:
                            pass
            res = bass_utils.run_bass_kernel_spmd(
                nc, in_maps, core_ids=list(range(N_CORES)), **kw)
        except Exception as e:
            last_exc = e
            time.sleep(5)
            continue
        ok = True
        for i in range(N_CORES):
            got = res.results[i]["out"][0].astype(np.float64) / 4096.0
            ref = checks[i]
            tol = 0.05 * max(np.abs(ref).max(), 1e-30)
            if not np.all(np.isfinite(got)) or np.abs(got - ref).max() > tol:
                ok = False
                break
        if ok:
            break
        time.sleep(5)
    else:
        if last_exc is not None:
            raise last_exc
    LAST_RESULTS = res

    out = np.concatenate([r["out"] for r in res.results], axis=0)
    out = out.astype(np.float32) * (1.0 / 4096.0)
    return out.reshape(B_TOTAL, C, S, S)
